# revision 1
# baseline (speedup 1.0000x reference)
"""Local (windowed) attention with shared KV head — TRN2 Bass kernel.

Problem: b=1, L=4096, d_model=1024, n_head=16, d_head=64, w=512.
  qp = (q@Wq)/8; k,v = kv@Wkv; per 512-chunk attention over {prev,self,next}
  chunks with zero-padded edges (softmax includes exp(0)=1 terms for pads);
  out = ctx @ Wo.

Sharding: sequence-parallel over the 8 chunks, one chunk per NeuronCore.
Each core recomputes the K/V projection for its 3-chunk halo (no
collectives). Edge cores receive zero-filled halo slices, which reproduces
the reference's zero-padding exactly (scores 0 -> exp 1 in the softmax).

Per-core dataflow (all matmuls in float32r = full-rate PE, ~1e-4 rel err):
  kvp^T = [Wv|Wk]^T @ kv^T            (24 MMs)   -> vT (rows 0:64), kT (64:128)
  k3T2  = kT duplicated to both partition halves (SBUF->SBUF DMA)
  v65   = PE-transpose(vT) with a ones column appended   ([y,64+1] tiles)
  qp^T  = (Wq/8)^T @ q^T              (64 MMs)   -> 8 tiles [128,512], head pair per tile
  scores: S^T[y,x] per head, row-packed pairs (2 heads share the PE array)
  P^T   = exp(S^T) on ScalarE, PSUM->SBUF, [128,1024] groups
  ctx^T+Z = [v|1]^T @ P^T fused       (M=65: rows 0:64 ctx, row 64 = softmax denom)
  norm  : zinv=1/Z; broadcast via K=1 matmul; ctxn = ctx * zinv_bcast
  out   = ctxn^T-tiles (lhsT) @ Wo    (64 MMs)   -> [512,1024] row-major -> DMA
"""

import numpy as np

B, L, DM, NH, DH, W = 1, 4096, 1024, 16, 64, 512
NCORES = 8
CH = L // NCORES        # 512 tokens per core
YW = 3 * W              # 1536 halo positions
P = 128
NF = DM // P            # 8 feature tiles
NY = YW // P            # 12 y tiles
NPAIR = NH // 2         # 8 head pairs
NGRP = NY // 2          # 6 score groups of 2 y-tiles

_CACHE = {}


def _build():
    import concourse.mybir as mybir
    import concourse.tile as tile
    from concourse import bacc
    from concourse.masks import make_identity
    from contextlib import ExitStack

    F32 = mybir.dt.float32
    F32R = mybir.dt.float32r
    EXP = mybir.ActivationFunctionType.Exp

    nc = bacc.Bacc("TRN2", target_bir_lowering=False, debug=False)
    QT = nc.dram_tensor("QT", [DM, CH], F32R, kind="ExternalInput")
    KVT = nc.dram_tensor("KVT", [DM, YW], F32R, kind="ExternalInput")
    WQ = nc.dram_tensor("WQ", [DM, DM], F32R, kind="ExternalInput")     # pre-scaled by 1/8
    WVK = nc.dram_tensor("WVK", [DM, P], F32R, kind="ExternalInput")    # [Wv | Wk]
    WO = nc.dram_tensor("WO", [DM, DM], F32R, kind="ExternalInput")
    OUT = nc.dram_tensor("OUT", [CH, DM], F32, kind="ExternalOutput")

    with tile.TileContext(nc) as tc, ExitStack() as ctx:
        perm = ctx.enter_context(tc.tile_pool(name="perm", bufs=1))

        identf = perm.tile([P, P], F32, tag="identf")
        make_identity(nc, identf[:])
        onesb = perm.tile([P, P], F32R, tag="onesb")
        nc.vector.memset(onesb[:].bitcast(F32), 1.0)

        # --- persistent SBUF tiles
        wvk = [perm.tile([P, P], F32R, tag=f"wvk{f}", name=f"wvk{f}") for f in range(NF)]
        wq = [perm.tile([P, DM], F32R, tag=f"wq{f}", name=f"wq{f}") for f in range(NF)]
        wo = [perm.tile([P, DM], F32R, tag=f"wo{f}", name=f"wo{f}") for f in range(NF)]
        k3T2 = perm.tile([P, YW], F32R, tag="k3T2")
        vTs = perm.tile([64, YW], F32, tag="vTs")
        v65 = [perm.tile([P, 65], F32R, tag=f"v65_{t}", name=f"v65_{t}") for t in range(NY)]
        qpT = [perm.tile([P, CH], F32R, tag=f"qpT{m}", name=f"qpT{m}") for m in range(NF)]
        ctxn = [perm.tile([P, CH], F32R, tag=f"ctxn{i}", name=f"ctxn{i}") for i in range(NPAIR)]

        for f in range(NF):
            nc.sync.dma_start(wvk[f][:], WVK.ap()[P * f:P * (f + 1), :])

        with tc.tile_pool(name="kvt", bufs=1) as kvtp, \
             tc.tile_pool(name="ph0ps", bufs=3, space="PSUM") as ph0, \
             tc.tile_pool(name="tpps", bufs=2, space="PSUM") as tpp:
            kvt = [kvtp.tile([P, YW], F32R, tag=f"kvt{f}", name=f"kvt{f}") for f in range(NF)]
            for f in range(NF):
                nc.sync.dma_start(kvt[f][:], KVT.ap()[P * f:P * (f + 1), :])
            # kv projection: [128,512] psum per n-tile; rows 0:64=vT, 64:128=kT
            for n in range(3):
                ps = ph0.tile([P, W], F32, tag="kvp")
                for f in range(NF):
                    nc.tensor.matmul(ps[:], wvk[f][:], kvt[f][:, W * n:W * (n + 1)],
                                     start=(f == 0), stop=(f == NF - 1))
                ns = slice(W * n, W * (n + 1))
                nc.vector.tensor_copy(vTs[:, ns], ps[0:64, :])
                nc.vector.tensor_copy(k3T2[64:128, ns], ps[64:128, :])
            # duplicate kT into the low partition half (partition remap DMA)
            nc.sync.dma_start(k3T2[0:64, :], k3T2[64:128, :])
            # v65 tiles: PE transpose of vT + ones column
            for t in range(NY):
                tp = tpp.tile([P, 64], F32, tag="tp")
                nc.tensor.transpose(tp[:], vTs[:, P * t:P * (t + 1)],
                                    identf[0:64, 0:64])
                nc.vector.tensor_copy(v65[t][:, 0:64], tp[:])
                nc.vector.memset(v65[t][:, 64:65].bitcast(F32), 1.0)

        # --- q projection
        with tc.tile_pool(name="qt", bufs=1) as qtp, \
             tc.tile_pool(name="qpps", bufs=8, space="PSUM") as qpp:
            qt = [qtp.tile([P, CH], F32R, tag=f"qt{f}", name=f"qt{f}") for f in range(NF)]
            for f in range(NF):
                nc.sync.dma_start(qt[f][:], QT.ap()[P * f:P * (f + 1), :])
            for f in range(NF):
                nc.sync.dma_start(wq[f][:], WQ.ap()[P * f:P * (f + 1), :])
            for m in range(NF):
                ps = qpp.tile([P, CH], F32, tag="qp")
                for f in range(NF):
                    nc.tensor.matmul(ps[:], wq[f][:, P * m:P * (m + 1)], qt[f][:],
                                     start=(f == 0), stop=(f == NF - 1))
                nc.vector.tensor_copy(qpT[m][:], ps[:])

        for f in range(NF):
            nc.sync.dma_start(wo[f][:], WO.ap()[P * f:P * (f + 1), :])

        # --- attention per head pair
        with tc.tile_pool(name="scps", bufs=2, space="PSUM") as scp, \
             tc.tile_pool(name="cxps", bufs=3, space="PSUM") as cxp, \
             tc.tile_pool(name="pt", bufs=4) as ptp, \
             tc.tile_pool(name="zn", bufs=4) as znp:
            for i in range(NPAIR):
                cxA = cxp.tile([P, W], F32, tag="cx")
                cxB = cxp.tile([P, W], F32, tag="cx")
                for g in range(NGRP):
                    scA = scp.tile([P, 2 * W], F32, tag="sc")
                    scB = scp.tile([P, 2 * W], F32, tag="sc")
                    for t in range(2):
                        y = 2 * g + t
                        ys = slice(P * y, P * (y + 1))
                        ts_ = slice(W * t, W * (t + 1))
                        nc.tensor.matmul(scA[:, ts_], k3T2[0:64, ys],
                                         qpT[i][0:64, :], start=True, stop=True,
                                         tile_position=(0, 0))
                        nc.tensor.matmul(scB[:, ts_], k3T2[64:128, ys],
                                         qpT[i][64:128, :], start=True, stop=True,
                                         tile_position=(64, 0))
                    pA = ptp.tile([P, 2 * W], F32R, tag="pt")
                    pB = ptp.tile([P, 2 * W], F32R, tag="pt")
                    nc.scalar.activation(pA[:], scA[:], EXP)
                    nc.scalar.activation(pB[:], scB[:], EXP)
                    for t in range(2):
                        y = 2 * g + t
                        ts_ = slice(W * t, W * (t + 1))
                        st = (g == 0 and t == 0)
                        sp = (g == NGRP - 1 and t == 1)
                        nc.tensor.matmul(cxA[0:65, :], v65[y][:], pA[:, ts_],
                                         start=st, stop=sp)
                        nc.tensor.matmul(cxB[0:65, :], v65[y][:], pB[:, ts_],
                                         start=st, stop=sp)
                # normalize: ctxn[i][0:64] = cxA/Z_A ; [64:128] = cxB/Z_B (via DMA)
                for h, cx in ((0, cxA), (1, cxB)):
                    zinv = znp.tile([65, W], F32R, tag="zinv")
                    with nc.allow_low_precision(reason="softmax denom feeds f32r matmul"):
                        nc.vector.reciprocal(zinv[64:65, :], cx[64:65, :])
                    zbc = cxp.tile([P, W], F32, tag="cx")
                    nc.tensor.matmul(zbc[0:64, :], onesb[64:65, 0:64],
                                     zinv[64:65, :], start=True, stop=True,
                                     tile_position=(64, 0))
                    cxs = znp.tile([64, W], F32, tag="cxs")
                    nc.vector.tensor_copy(cxs[:], cx[0:64, :])
                    if h == 0:
                        with nc.allow_low_precision(reason="ctx feeds f32r matmul"):
                            nc.vector.tensor_mul(ctxn[i][0:64, :], cxs[:],
                                                 zbc[0:64, :])
                    else:
                        cbt = znp.tile([64, W], F32R, tag="cbt")
                        with nc.allow_low_precision(reason="ctx feeds f32r matmul"):
                            nc.vector.tensor_mul(cbt[:], cxs[:], zbc[0:64, :])
                        nc.sync.dma_start(ctxn[i][64:128, :], cbt[:])

        # --- output projection: out[x,o] = sum_i ctxn[i][:,x].T @ wo[i][:,o]
        with tc.tile_pool(name="opps", bufs=8, space="PSUM") as opp, \
             tc.tile_pool(name="osb", bufs=4) as osb:
            for x in range(4):
                xs = slice(P * x, P * (x + 1))
                for o in range(2):
                    os_ = slice(W * o, W * (o + 1))
                    ps = opp.tile([P, W], F32, tag="op")
                    for i in range(NPAIR):
                        nc.tensor.matmul(ps[:], ctxn[i][:, xs], wo[i][:, os_],
                                         start=(i == 0), stop=(i == NPAIR - 1))
                    ot = osb.tile([P, W], F32, tag="os")
                    nc.scalar.copy(ot[:], ps[:])
                    nc.sync.dma_start(OUT.ap()[xs, os_], ot[:])

    nc.compile()
    return nc


def _get_nc():
    if "nc" not in _CACHE:
        _CACHE["nc"] = _build()
    return _CACHE["nc"]


def kernel(q, kv, Wq, Wkv, Wo, w=None, _trace=False):
    from concourse import bass_utils

    q = np.asarray(q, np.float32).reshape(L, DM)
    kv = np.asarray(kv, np.float32).reshape(L, DM)
    Wq = np.asarray(Wq, np.float32)
    Wkv = np.asarray(Wkv, np.float32)
    Wo = np.asarray(Wo, np.float32)

    qT = np.ascontiguousarray(q.T)                      # [DM, L]
    kvT = np.ascontiguousarray(kv.T)                    # [DM, L]
    WQs = np.ascontiguousarray(Wq / np.sqrt(DH))        # fold 1/sqrt(d_head)
    WVK = np.ascontiguousarray(
        np.concatenate([Wkv[:, DH:], Wkv[:, :DH]], axis=1))  # [Wv | Wk]

    in_maps = []
    for c in range(NCORES):
        kvt_c = np.zeros((DM, YW), np.float32)
        lo = (c - 1) * CH
        hi = (c + 2) * CH
        src_lo, src_hi = max(lo, 0), min(hi, L)
        dst_lo = src_lo - lo
        kvt_c[:, dst_lo:dst_lo + (src_hi - src_lo)] = kvT[:, src_lo:src_hi]
        in_maps.append({
            "QT": np.ascontiguousarray(qT[:, c * CH:(c + 1) * CH]),
            "KVT": kvt_c,
            "WQ": WQs,
            "WVK": WVK,
            "WO": Wo,
        })

    nc = _get_nc()
    res = bass_utils.run_bass_kernel_spmd(
        nc, in_maps, core_ids=list(range(NCORES)), trace=_trace)
    if _trace:
        _CACHE["last_result"] = res

    out = np.concatenate([r["OUT"] for r in res.results], axis=0)
    return out.reshape(B, L, DM).astype(np.float32)



# revision 8
# speedup vs baseline: 1.1634x; 1.1634x over previous
"""Local (windowed) attention with shared KV head — TRN2 Bass kernel.

Problem: b=1, L=4096, d_model=1024, n_head=16, d_head=64, w=512.
  qp = (q@Wq)/8; k,v = kv@Wkv; per 512-chunk attention over {prev,self,next}
  chunks with zero-padded edges (softmax includes exp(0)=1 terms for pads);
  out = ctx @ Wo.

Sharding: sequence-parallel over the 8 chunks, one chunk per NeuronCore.
Each core recomputes the K/V projection for its 3-chunk halo (no
collectives). Edge cores receive zero-filled halo slices, which reproduces
the reference's zero-padding exactly (scores 0 -> exp 1 in the softmax).

All matmuls in bf16 (1 cycle/row on the PE at 2.4 GHz vs ~1.5 for the
fp32 path, and bf16 activity keeps the HAM clock gate open). PSUM
accumulation stays fp32. Softmax exp runs on ScalarE reading PSUM
directly and writing bf16 probs to SBUF.

Per-core dataflow:
  kvp^T = [Wv|Wk]^T @ kv^T            (24 MMs)   -> vTs (rows 0:64), kT (64:128)
  k3T2  = kT duplicated to both partition halves (SBUF->SBUF DMA)
  v65   = PE-transpose(vTs) with a ones column appended   ([y,64+1] tiles)
  qp^T  = (Wq/8)^T @ q^T              (64 MMs)   -> 8 tiles [128,512] bf16
  scores: S^T[y,x] per head, row-packed pairs (2 heads in PE row halves)
  P^T   = exp(S^T) on ScalarE, PSUM->SBUF bf16, [128,1024] tiles
  ctx^T+Z = [v|1]^T @ P^T fused       (M=65: rows 0:64 ctx, row 64 = denom)
  norm  : zinv = recip_approx_fast(Z); broadcast via K=1 matmul; ctx * zinv
  out   = ctxn^T-tiles (lhsT) @ Wo    (64 MMs)   -> [512,1024] f32 -> DMA
"""

import numpy as np

B, L, DM, NH, DH, W = 1, 4096, 1024, 16, 64, 512
NCORES = 8
CH = L // NCORES        # 512 tokens per core
YW = 3 * W              # 1536 halo positions
P = 128
NF = DM // P            # 8 feature tiles
NY = YW // P            # 12 y tiles
NPAIR = NH // 2         # 8 head pairs
NGRP = NY // 2          # 6 score groups of 2 y-tiles

_CACHE = {}


def _build():
    import concourse.mybir as mybir
    import concourse.tile as tile
    from concourse import bacc
    from concourse.masks import make_identity
    from contextlib import ExitStack

    F32 = mybir.dt.float32
    BF16 = mybir.dt.bfloat16
    EXP = mybir.ActivationFunctionType.Exp

    nc = bacc.Bacc("TRN2", target_bir_lowering=False, debug=False)
    QT = nc.dram_tensor("QT", [DM, CH], BF16, kind="ExternalInput")
    KVT = nc.dram_tensor("KVT", [DM, YW], BF16, kind="ExternalInput")
    WQ = nc.dram_tensor("WQ", [DM, DM], BF16, kind="ExternalInput")    # pre-scaled by 1/8
    WVK = nc.dram_tensor("WVK", [DM, P], BF16, kind="ExternalInput")   # [Wv | Wk]
    WO = nc.dram_tensor("WO", [DM, DM], BF16, kind="ExternalInput")
    OUT = nc.dram_tensor("OUT", [CH, DM], F32, kind="ExternalOutput")

    with tile.TileContext(nc) as tc, ExitStack() as ctx:
        perm = ctx.enter_context(tc.tile_pool(name="perm", bufs=1))

        identb = perm.tile([64, 64], F32, tag="identb")
        make_identity(nc, identb[:])
        onesb = perm.tile([P, 64], BF16, tag="onesb")
        nc.vector.memset(onesb[:], 1.0)

        # --- persistent SBUF tiles (all bf16)
        wvk = [perm.tile([P, P], BF16, tag=f"wvk{f}", name=f"wvk{f}") for f in range(NF)]
        wq = [perm.tile([P, DM], BF16, tag=f"wq{f}", name=f"wq{f}") for f in range(NF)]
        wo = [perm.tile([P, DM], BF16, tag=f"wo{f}", name=f"wo{f}") for f in range(NF)]
        k3T2 = perm.tile([P, YW], BF16, tag="k3T2")
        vTs = perm.tile([64, YW], F32, tag="vTs")
        v65 = [perm.tile([P, 65], BF16, tag=f"v65_{t}", name=f"v65_{t}") for t in range(NY)]
        qpT = [perm.tile([P, CH], BF16, tag=f"qpT{m}", name=f"qpT{m}") for m in range(NF)]
        ctxn = [perm.tile([P, CH], BF16, tag=f"ctxn{i}", name=f"ctxn{i}") for i in range(NPAIR)]

        for f in range(NF):
            nc.sync.dma_start(wvk[f][:], WVK.ap()[P * f:P * (f + 1), :])

        with tc.tile_pool(name="kvt", bufs=1) as kvtp, \
             tc.tile_pool(name="ph0ps", bufs=3, space="PSUM") as ph0, \
             tc.tile_pool(name="tpps", bufs=2, space="PSUM") as tpp:
            kvt = [kvtp.tile([P, YW], BF16, tag=f"kvt{f}", name=f"kvt{f}") for f in range(NF)]
            for f in range(NF):
                nc.sync.dma_start(kvt[f][:], KVT.ap()[P * f:P * (f + 1), :])
            # kv projection: [128,512] psum per n-tile; rows 0:64=vT, 64:128=kT
            for n in range(3):
                ps = ph0.tile([P, W], F32, tag="kvp")
                for f in range(NF):
                    nc.tensor.matmul(ps[:], wvk[f][:], kvt[f][:, W * n:W * (n + 1)],
                                     start=(f == 0), stop=(f == NF - 1))
                ns = slice(W * n, W * (n + 1))
                with nc.allow_low_precision(reason="bf16 attention pipeline"):
                    nc.vector.tensor_copy(vTs[:, ns], ps[0:64, :])
                    nc.vector.tensor_copy(k3T2[64:128, ns], ps[64:128, :])
            # (vTs stays f32: the PE transpose requires out dtype == in dtype)
            # duplicate kT into the low partition half (partition remap DMA)
            nc.sync.dma_start(k3T2[0:64, :], k3T2[64:128, :])
            # v65 tiles: PE transpose of vT + ones column
            for t in range(NY):
                tp = tpp.tile([P, 64], F32, tag="tp")
                nc.tensor.transpose(tp[:], vTs[:, P * t:P * (t + 1)], identb[:])
                with nc.allow_low_precision(reason="bf16 attention pipeline"):
                    nc.vector.tensor_copy(v65[t][:, 0:64], tp[:])
                nc.vector.memset(v65[t][:, 64:65], 1.0)

        # --- q projection (1 psum bank, overlaps the attention phase) + attention
        with tc.tile_pool(name="qt", bufs=1) as qtp, \
             tc.tile_pool(name="qpps", bufs=1, space="PSUM") as qpp, \
             tc.tile_pool(name="scps", bufs=2, space="PSUM") as scp, \
             tc.tile_pool(name="cxps", bufs=3, space="PSUM") as cxp, \
             tc.tile_pool(name="pt", bufs=4) as ptp, \
             tc.tile_pool(name="zn", bufs=6) as znp:
            qt = [qtp.tile([P, CH], BF16, tag=f"qt{f}", name=f"qt{f}") for f in range(NF)]
            for f in range(NF):
                nc.sync.dma_start(qt[f][:], QT.ap()[P * f:P * (f + 1), :])
            for f in range(NF):
                nc.sync.dma_start(wq[f][:], WQ.ap()[P * f:P * (f + 1), :])
            for m in range(NF):
                ps = qpp.tile([P, CH], F32, tag="qp")
                for f in range(NF):
                    nc.tensor.matmul(ps[:], wq[f][:, P * m:P * (m + 1)], qt[f][:],
                                     start=(f == 0), stop=(f == NF - 1))
                with nc.allow_low_precision(reason="bf16 attention pipeline"):
                    nc.vector.tensor_copy(qpT[m][:], ps[:])

            for f in range(NF):
                nc.sync.dma_start(wo[f][:], WO.ap()[P * f:P * (f + 1), :])

            # --- attention per head pair
            for i in range(NPAIR):
                cxA = cxp.tile([P, W], F32, tag="cx")
                cxB = cxp.tile([P, W], F32, tag="cx")
                for g in range(NGRP):
                    scA = scp.tile([P, 2 * W], F32, tag="sc")
                    scB = scp.tile([P, 2 * W], F32, tag="sc")
                    for t in range(2):
                        y = 2 * g + t
                        ys = slice(P * y, P * (y + 1))
                        ts_ = slice(W * t, W * (t + 1))
                        nc.tensor.matmul(scA[:, ts_], k3T2[0:64, ys],
                                         qpT[i][0:64, :], start=True, stop=True,
                                         tile_position=(0, 0))
                        nc.tensor.matmul(scB[:, ts_], k3T2[64:128, ys],
                                         qpT[i][64:128, :], start=True, stop=True,
                                         tile_position=(64, 0))
                    pA = ptp.tile([P, 2 * W], BF16, tag="pt")
                    pB = ptp.tile([P, 2 * W], BF16, tag="pt")
                    with nc.allow_low_precision(reason="bf16 probs"):
                        nc.scalar.activation(pA[:], scA[:], EXP)
                        nc.scalar.activation(pB[:], scB[:], EXP)
                    for t in range(2):
                        y = 2 * g + t
                        ts_ = slice(W * t, W * (t + 1))
                        st = (g == 0 and t == 0)
                        sp = (g == NGRP - 1 and t == 1)
                        nc.tensor.matmul(cxA[0:65, :], v65[y][:], pA[:, ts_],
                                         start=st, stop=sp)
                        nc.tensor.matmul(cxB[0:65, :], v65[y][:], pB[:, ts_],
                                         start=st, stop=sp)
                # normalize: ctxn[i][0:64] = cxA/Z_A ; [64:128] = cxB/Z_B (via DMA)
                for h, cx in ((0, cxA), (1, cxB)):
                    zt = znp.tile([65, W], F32, tag="zt")
                    with nc.allow_low_precision(reason="softmax denom"):
                        nc.vector.reciprocal(zt[64:65, :], cx[64:65, :])
                    ztb = znp.tile([65, W], BF16, tag="ztb")
                    with nc.allow_low_precision(reason="softmax denom"):
                        nc.vector.tensor_copy(ztb[64:65, :], zt[64:65, :])
                    cxs = znp.tile([64, W], BF16, tag="cxs")
                    with nc.allow_low_precision(reason="bf16 ctx"):
                        nc.vector.tensor_copy(cxs[:], cx[0:64, :])
                    zb = cxp.tile([P, W], F32, tag="cx")
                    nc.tensor.matmul(zb[0:64, :], onesb[64:65, 0:64],
                                     ztb[64:65, :], start=True, stop=True,
                                     tile_position=(64, 0))
                    if h == 0:
                        with nc.allow_low_precision(reason="bf16 ctx"):
                            nc.vector.tensor_mul(ctxn[i][0:64, :], cxs[:],
                                                 zb[0:64, :])
                    else:
                        cbt = znp.tile([64, W], BF16, tag="cbt")
                        with nc.allow_low_precision(reason="bf16 ctx"):
                            nc.vector.tensor_mul(cbt[:], cxs[:], zb[0:64, :])
                        nc.sync.dma_start(ctxn[i][64:128, :], cbt[:])

        # --- output projection: out[x,o] = sum_i ctxn[i][:,x].T @ wo[i][:,o]
        with tc.tile_pool(name="opps", bufs=8, space="PSUM") as opp, \
             tc.tile_pool(name="osb", bufs=4) as osb:
            for x in range(4):
                xs = slice(P * x, P * (x + 1))
                for o in range(2):
                    os_ = slice(W * o, W * (o + 1))
                    ps = opp.tile([P, W], F32, tag="op")
                    for i in range(NPAIR):
                        nc.tensor.matmul(ps[:], ctxn[i][:, xs], wo[i][:, os_],
                                         start=(i == 0), stop=(i == NPAIR - 1))
                    ot = osb.tile([P, W], F32, tag="os")
                    nc.scalar.copy(ot[:], ps[:])
                    nc.sync.dma_start(OUT.ap()[xs, os_], ot[:])

    nc.compile()
    return nc


def _get_nc():
    if "nc" not in _CACHE:
        _CACHE["nc"] = _build()
    return _CACHE["nc"]


def kernel(q, kv, Wq, Wkv, Wo, w=None, _trace=False):
    from concourse import bass_utils
    import ml_dtypes

    BF = ml_dtypes.bfloat16

    q = np.asarray(q, np.float32).reshape(L, DM)
    kv = np.asarray(kv, np.float32).reshape(L, DM)
    Wq = np.asarray(Wq, np.float32)
    Wkv = np.asarray(Wkv, np.float32)
    Wo = np.asarray(Wo, np.float32)

    qT = np.ascontiguousarray(q.T).astype(BF)           # [DM, L]
    kvT = np.ascontiguousarray(kv.T).astype(BF)         # [DM, L]
    WQs = np.ascontiguousarray(Wq / np.sqrt(DH)).astype(BF)   # fold 1/sqrt(d_head)
    WVK = np.ascontiguousarray(
        np.concatenate([Wkv[:, DH:], Wkv[:, :DH]], axis=1)).astype(BF)  # [Wv | Wk]
    WOb = np.ascontiguousarray(Wo).astype(BF)

    in_maps = []
    for c in range(NCORES):
        kvt_c = np.zeros((DM, YW), BF)
        lo = (c - 1) * CH
        hi = (c + 2) * CH
        src_lo, src_hi = max(lo, 0), min(hi, L)
        dst_lo = src_lo - lo
        kvt_c[:, dst_lo:dst_lo + (src_hi - src_lo)] = kvT[:, src_lo:src_hi]
        in_maps.append({
            "QT": np.ascontiguousarray(qT[:, c * CH:(c + 1) * CH]),
            "KVT": kvt_c,
            "WQ": WQs,
            "WVK": WVK,
            "WO": WOb,
        })

    nc = _get_nc()
    res = bass_utils.run_bass_kernel_spmd(
        nc, in_maps, core_ids=list(range(NCORES)), trace=_trace)
    if _trace:
        _CACHE["last_result"] = res

    out = np.concatenate([r["OUT"] for r in res.results], axis=0)
    return out.reshape(B, L, DM).astype(np.float32)


# revision 19
# speedup vs baseline: 1.4627x; 1.2573x over previous
"""Local (windowed) attention with shared KV head — TRN2 Bass kernel.

Problem: b=1, L=4096, d_model=1024, n_head=16, d_head=64, w=512.
  qp = (q@Wq)/8; k,v = kv@Wkv; per 512-chunk attention over {prev,self,next}
  chunks with zero-padded edges (softmax includes exp(0)=1 terms for pads);
  out = ctx @ Wo.

Sharding: sequence-parallel over the 8 chunks, one chunk per NeuronCore.
Each core recomputes the K/V projection for its 3-chunk halo (no
collectives). Edge cores receive zero-filled halo slices, which reproduces
the reference's zero-padding exactly (scores 0 -> exp 1 in the softmax).

All matmuls in bf16 (1 cycle/row on the PE at 2.4 GHz vs ~1.5 for the
fp32 path, and bf16 activity keeps the HAM clock gate open). PSUM
accumulation stays fp32. Softmax exp runs on ScalarE reading PSUM
directly and writing bf16 probs to SBUF.

Per-core dataflow:
  kvp^T = [Wv|Wk]^T @ kv^T            (24 MMs)   -> vTs (rows 0:64), kT (64:128)
  k3T2  = kT duplicated to both partition halves (SBUF->SBUF DMA)
  v65   = PE-transpose(vTs) with a ones column appended   ([y,64+1] tiles)
  qp^T  = (Wq/8)^T @ q^T              (64 MMs)   -> 8 tiles [128,512] bf16
  scores: S^T[y,x] per head, row-packed pairs (2 heads in PE row halves)
  P^T   = exp(S^T) on ScalarE, PSUM->SBUF bf16, [128,1024] tiles
  ctx^T+Z = [v|1]^T @ P^T fused       (M=65: rows 0:64 ctx, row 64 = denom)
  norm  : zinv = recip_approx_fast(Z); broadcast via K=1 matmul; ctx * zinv
  out   = ctxn^T-tiles (lhsT) @ Wo    (64 MMs)   -> [512,1024] f32 -> DMA
"""

import numpy as np

B, L, DM, NH, DH, W = 1, 4096, 1024, 16, 64, 512
NCORES = 8
CH = L // NCORES        # 512 tokens per core
YW = 3 * W              # 1536 halo positions
P = 128
NF = DM // P            # 8 feature tiles
NY = YW // P            # 12 y tiles
NPAIR = NH // 2         # 8 head pairs
NGRP = NY // 2          # 6 score groups of 2 y-tiles

_CACHE = {}


def _build():
    import concourse.mybir as mybir
    import concourse.tile as tile
    from concourse import bacc
    from concourse.masks import make_identity
    from contextlib import ExitStack

    F32 = mybir.dt.float32
    BF16 = mybir.dt.bfloat16
    EXP = mybir.ActivationFunctionType.Exp

    nc = bacc.Bacc("TRN2", target_bir_lowering=False, debug=False)
    QT = nc.dram_tensor("QT", [DM, CH], BF16, kind="ExternalInput")
    ESEL = nc.dram_tensor("ESEL", [16, NH * 64], BF16, kind="ExternalInput")
    KVT = nc.dram_tensor("KVT", [DM, YW], BF16, kind="ExternalInput")
    WQ = nc.dram_tensor("WQ", [DM, DM], BF16, kind="ExternalInput")    # pre-scaled by 1/8
    WVK = nc.dram_tensor("WVK", [DM, P], BF16, kind="ExternalInput")   # [Wv | Wk]
    WO = nc.dram_tensor("WO", [DM, DM], BF16, kind="ExternalInput")
    OUT = nc.dram_tensor("OUT", [CH, DM], F32, kind="ExternalOutput")

    with tile.TileContext(nc) as tc, ExitStack() as ctx:
        perm = ctx.enter_context(tc.tile_pool(name="perm", bufs=1))

        identb = perm.tile([64, 64], F32, tag="identb")
        make_identity(nc, identb[:])
        # E[h', 64h:64h+64] = 1 iff h'==h : K=16 one-hot selector for the
        # zinv broadcast matmul (host-provided constant)
        esel = perm.tile([16, NH * 64], BF16, tag="esel")
        nc.sync.dma_start(esel[:], ESEL.ap()[:, :])

        # --- persistent SBUF tiles (all bf16)
        wvk = [perm.tile([P, P], BF16, tag=f"wvk{f}", name=f"wvk{f}") for f in range(NF)]
        wq = [perm.tile([P, DM], BF16, tag=f"wq{f}", name=f"wq{f}") for f in range(NF)]
        wo = [perm.tile([P, DM], BF16, tag=f"wo{f}", name=f"wo{f}") for f in range(NF)]
        k3T2 = perm.tile([P, YW], BF16, tag="k3T2")
        vTs = perm.tile([64, YW], F32, tag="vTs")
        v65 = [perm.tile([P, 65], BF16, tag=f"v65_{t}", name=f"v65_{t}") for t in range(NY)]
        qpT = [perm.tile([P, CH], BF16, tag=f"qpT{m}", name=f"qpT{m}") for m in range(NF)]
        ctxn = [perm.tile([P, CH], BF16, tag=f"ctxn{i}", name=f"ctxn{i}") for i in range(NPAIR)]
        cxs = [perm.tile([64, W], BF16, tag=f"cxs{h}", name=f"cxs{h}") for h in range(NH)]
        zr16 = perm.tile([16, W], F32, tag="zr16")
        zi16 = perm.tile([16, W], F32, tag="zi16")
        zi16b = perm.tile([16, W], BF16, tag="zi16b")

        for f in range(NF):
            nc.sync.dma_start(wvk[f][:], WVK.ap()[P * f:P * (f + 1), :])

        with tc.tile_pool(name="kvt", bufs=1) as kvtp, \
             tc.tile_pool(name="ph0ps", bufs=3, space="PSUM") as ph0, \
             tc.tile_pool(name="tpps", bufs=2, space="PSUM") as tpp:
            kvt = [kvtp.tile([P, YW], BF16, tag=f"kvt{f}", name=f"kvt{f}") for f in range(NF)]
            # split loads per w-chunk so the first kv-proj matmuls start early
            for n in range(3):
                for f in range(NF):
                    ns_ = slice(W * n, W * (n + 1))
                    nc.sync.dma_start(kvt[f][:, ns_], KVT.ap()[P * f:P * (f + 1), ns_])
            # kv projection: [128,512] psum per n-tile; rows 0:64=vT, 64:128=kT
            for n in range(3):
                ps = ph0.tile([P, W], F32, tag="kvp")
                for f in range(NF):
                    nc.tensor.matmul(ps[:], wvk[f][:], kvt[f][:, W * n:W * (n + 1)],
                                     start=(f == 0), stop=(f == NF - 1))
                ns = slice(W * n, W * (n + 1))
                with nc.allow_low_precision(reason="bf16 attention pipeline"):
                    nc.vector.tensor_copy(vTs[:, ns], ps[0:64, :])
                    nc.vector.tensor_copy(k3T2[64:128, ns], ps[64:128, :])
            # (vTs stays f32: the PE transpose requires out dtype == in dtype)
            # duplicate kT into the low partition half (partition remap DMA)
            nc.sync.dma_start(k3T2[0:64, :], k3T2[64:128, :])
            # v65 tiles: PE transpose of vT + ones column
            for t in range(NY):
                tp = tpp.tile([P, 64], F32, tag="tp")
                nc.tensor.transpose(tp[:], vTs[:, P * t:P * (t + 1)], identb[:])
                with nc.allow_low_precision(reason="bf16 attention pipeline"):
                    nc.vector.tensor_copy(v65[t][:, 0:64], tp[:])
                nc.vector.memset(v65[t][:, 64:65], 1.0)

        # --- q projection (1 psum bank, overlaps the attention phase) + attention
        with tc.tile_pool(name="qt", bufs=1) as qtp, \
             tc.tile_pool(name="qpps", bufs=1, space="PSUM") as qpp, \
             tc.tile_pool(name="scps", bufs=2, space="PSUM") as scp, \
             tc.tile_pool(name="cxps", bufs=3, space="PSUM") as cxp, \
             tc.tile_pool(name="pt", bufs=4) as ptp, \
             tc.tile_pool(name="zn", bufs=6) as znp:
            qt = [qtp.tile([P, CH], BF16, tag=f"qt{f}", name=f"qt{f}") for f in range(NF)]
            for f in range(NF):
                nc.sync.dma_start(qt[f][:], QT.ap()[P * f:P * (f + 1), :])
            for h2 in range(2):
                hs = slice(W * h2, W * (h2 + 1))
                for f in range(NF):
                    nc.sync.dma_start(wq[f][:, hs], WQ.ap()[P * f:P * (f + 1), hs])
            for m in range(NF):
                ps = qpp.tile([P, CH], F32, tag="qp")
                for f in range(NF):
                    nc.tensor.matmul(ps[:], wq[f][:, P * m:P * (m + 1)], qt[f][:],
                                     start=(f == 0), stop=(f == NF - 1))
                with nc.allow_low_precision(reason="bf16 attention pipeline"):
                    nc.vector.tensor_copy(qpT[m][:], ps[:])

            for f in range(NF):
                nc.sync.dma_start(wo[f][:], WO.ap()[P * f:P * (f + 1), :])

            # --- attention per head pair
            for i in range(NPAIR):
                cxA = cxp.tile([P, W], F32, tag="cx")
                cxB = cxp.tile([P, W], F32, tag="cx")
                for g in range(NGRP):
                    scA = scp.tile([P, 2 * W], F32, tag="sc")
                    scB = scp.tile([P, 2 * W], F32, tag="sc")
                    for t in range(2):
                        y = 2 * g + t
                        ys = slice(P * y, P * (y + 1))
                        ts_ = slice(W * t, W * (t + 1))
                        nc.tensor.matmul(scA[:, ts_], k3T2[0:64, ys],
                                         qpT[i][0:64, :], start=True, stop=True,
                                         tile_position=(0, 0))
                        nc.tensor.matmul(scB[:, ts_], k3T2[64:128, ys],
                                         qpT[i][64:128, :], start=True, stop=True,
                                         tile_position=(64, 0))
                    pA = ptp.tile([P, 2 * W], BF16, tag="pt")
                    pB = ptp.tile([P, 2 * W], BF16, tag="pt")
                    with nc.allow_low_precision(reason="bf16 probs"):
                        nc.scalar.activation(pA[:], scA[:], EXP)
                        nc.scalar.activation(pB[:], scB[:], EXP)
                    for t in range(2):
                        y = 2 * g + t
                        ts_ = slice(W * t, W * (t + 1))
                        st = (g == 0 and t == 0)
                        sp = (g == NGRP - 1 and t == 1)
                        nc.tensor.matmul(cxA[0:65, :], v65[y][:], pA[:, ts_],
                                         start=st, stop=sp)
                        nc.tensor.matmul(cxB[0:65, :], v65[y][:], pB[:, ts_],
                                         start=st, stop=sp)
                # stage Z row + unnormalized ctx out of PSUM (frees cx banks);
                # the reciprocal + normalization run batched after all pairs
                for h, cx in ((0, cxA), (1, cxB)):
                    zt = znp.tile([65, W], F32, tag="zt")
                    nc.vector.tensor_copy(zt[64:65, :], cx[64:65, :])
                    nc.sync.dma_start(zr16[2 * i + h:2 * i + h + 1, :],
                                      zt[64:65, :])
                    with nc.allow_low_precision(reason="bf16 ctx"):
                        nc.vector.tensor_copy(cxs[2 * i + h][:], cx[0:64, :])

            # --- batched softmax denominators + normalization
            with nc.allow_low_precision(reason="softmax denom"):
                nc.vector.reciprocal(zi16[:], zr16[:])
                nc.vector.tensor_copy(zi16b[:], zi16[:])
            for i in range(NPAIR):
                for h in range(2):
                    hh = 2 * i + h
                    zb = qpp.tile([P, W], F32, tag="qp")
                    nc.tensor.matmul(zb[0:64, :], esel[:, 64 * hh:64 * (hh + 1)],
                                     zi16b[:], start=True, stop=True)
                    if h == 0:
                        with nc.allow_low_precision(reason="bf16 ctx"):
                            nc.vector.tensor_mul(ctxn[i][0:64, :], cxs[hh][:],
                                                 zb[0:64, :])
                    else:
                        cbt = znp.tile([64, W], BF16, tag="cbt")
                        with nc.allow_low_precision(reason="bf16 ctx"):
                            nc.vector.tensor_mul(cbt[:], cxs[hh][:], zb[0:64, :])
                        nc.sync.dma_start(ctxn[i][64:128, :], cbt[:])

        # --- output projection: out[x,o] = sum_i ctxn[i][:,x].T @ wo[i][:,o]
        with tc.tile_pool(name="opps", bufs=8, space="PSUM") as opp, \
             tc.tile_pool(name="osb", bufs=4) as osb:
            for x in range(4):
                xs = slice(P * x, P * (x + 1))
                for o in range(2):
                    os_ = slice(W * o, W * (o + 1))
                    ps = opp.tile([P, W], F32, tag="op")
                    for i in range(NPAIR):
                        nc.tensor.matmul(ps[:], ctxn[i][:, xs], wo[i][:, os_],
                                         start=(i == 0), stop=(i == NPAIR - 1))
                    ot = osb.tile([P, W], F32, tag="os")
                    nc.scalar.copy(ot[:], ps[:])
                    nc.sync.dma_start(OUT.ap()[xs, os_], ot[:])

    nc.compile()
    return nc


def _get_nc():
    if "nc" not in _CACHE:
        _CACHE["nc"] = _build()
    return _CACHE["nc"]


def _esel():
    import ml_dtypes
    e = np.zeros((16, NH * 64), ml_dtypes.bfloat16)
    for h in range(NH):
        e[h, 64 * h:64 * (h + 1)] = 1.0
    return e


def kernel(q, kv, Wq, Wkv, Wo, w=None, _trace=False):
    from concourse import bass_utils
    import ml_dtypes

    BF = ml_dtypes.bfloat16

    q = np.asarray(q, np.float32).reshape(L, DM)
    kv = np.asarray(kv, np.float32).reshape(L, DM)
    Wq = np.asarray(Wq, np.float32)
    Wkv = np.asarray(Wkv, np.float32)
    Wo = np.asarray(Wo, np.float32)

    qT = np.ascontiguousarray(q.T).astype(BF)           # [DM, L]
    kvT = np.ascontiguousarray(kv.T).astype(BF)         # [DM, L]
    WQs = np.ascontiguousarray(Wq / np.sqrt(DH)).astype(BF)   # fold 1/sqrt(d_head)
    WVK = np.ascontiguousarray(
        np.concatenate([Wkv[:, DH:], Wkv[:, :DH]], axis=1)).astype(BF)  # [Wv | Wk]
    WOb = np.ascontiguousarray(Wo).astype(BF)

    in_maps = []
    for c in range(NCORES):
        kvt_c = np.zeros((DM, YW), BF)
        lo = (c - 1) * CH
        hi = (c + 2) * CH
        src_lo, src_hi = max(lo, 0), min(hi, L)
        dst_lo = src_lo - lo
        kvt_c[:, dst_lo:dst_lo + (src_hi - src_lo)] = kvT[:, src_lo:src_hi]
        in_maps.append({
            "QT": np.ascontiguousarray(qT[:, c * CH:(c + 1) * CH]),
            "KVT": kvt_c,
            "WQ": WQs,
            "WVK": WVK,
            "WO": WOb,
            "ESEL": _esel(),
        })

    nc = _get_nc()
    res = bass_utils.run_bass_kernel_spmd(
        nc, in_maps, core_ids=list(range(NCORES)), trace=_trace)
    if _trace:
        _CACHE["last_result"] = res

    out = np.concatenate([r["OUT"] for r in res.results], axis=0)
    return out.reshape(B, L, DM).astype(np.float32)


# revision 25
# speedup vs baseline: 1.5023x; 1.0271x over previous
"""Local (windowed) attention with shared KV head — TRN2 Bass kernel.

Problem: b=1, L=4096, d_model=1024, n_head=16, d_head=64, w=512.
  qp = (q@Wq)/8; k,v = kv@Wkv; per 512-chunk attention over {prev,self,next}
  chunks with zero-padded edges (softmax includes exp(0)=1 terms for pads);
  out = ctx @ Wo.

Sharding: sequence-parallel over the 8 chunks, one chunk per NeuronCore.
Each core recomputes the K/V projection for its 3-chunk halo (no
collectives). Edge cores receive zero-filled halo slices, which reproduces
the reference's zero-padding exactly (scores 0 -> exp 1 in the softmax).

All matmuls in bf16 (1 cycle/row on the PE at 2.4 GHz vs ~1.5 for the
fp32 path, and bf16 activity keeps the HAM clock gate open). PSUM
accumulation stays fp32. Softmax exp runs on ScalarE reading PSUM
directly and writing bf16 probs to SBUF.

Per-core dataflow:
  kvp^T = [Wv|Wk]^T @ kv^T            (24 MMs)   -> vTs (rows 0:64), kT (64:128)
  k3T2  = kT duplicated to both partition halves (SBUF->SBUF DMA)
  v65   = PE-transpose(vTs) with a ones column appended   ([y,64+1] tiles)
  qp^T  = (Wq/8)^T @ q^T              (64 MMs)   -> 8 tiles [128,512] bf16
  scores: S^T[y,x] per head, row-packed pairs (2 heads in PE row halves)
  P^T   = exp(S^T) on ScalarE, PSUM->SBUF bf16, [128,1024] tiles
  ctx^T+Z = [v|1]^T @ P^T fused       (M=65: rows 0:64 ctx, row 64 = denom)
  norm  : zinv = recip_approx_fast(Z); broadcast via K=1 matmul; ctx * zinv
  out   = ctxn^T-tiles (lhsT) @ Wo    (64 MMs)   -> [512,1024] f32 -> DMA
"""

import numpy as np

B, L, DM, NH, DH, W = 1, 4096, 1024, 16, 64, 512
NCORES = 8
CH = L // NCORES        # 512 tokens per core
YW = 3 * W              # 1536 halo positions
P = 128
NF = DM // P            # 8 feature tiles
NY = YW // P            # 12 y tiles
NPAIR = NH // 2         # 8 head pairs
NGRP = NY // 2          # 6 score groups of 2 y-tiles

_CACHE = {}


def _build():
    import concourse.mybir as mybir
    import concourse.tile as tile
    from concourse import bacc
    from concourse.masks import make_identity
    from contextlib import ExitStack

    F32 = mybir.dt.float32
    BF16 = mybir.dt.bfloat16
    EXP = mybir.ActivationFunctionType.Exp

    nc = bacc.Bacc("TRN2", target_bir_lowering=False, debug=False)
    QT = nc.dram_tensor("QT", [DM, CH], BF16, kind="ExternalInput")
    ESEL = nc.dram_tensor("ESEL", [34, NH * 64], BF16, kind="ExternalInput")
    KVT = nc.dram_tensor("KVT", [DM, YW], BF16, kind="ExternalInput")
    WQ = nc.dram_tensor("WQ", [DM, DM], BF16, kind="ExternalInput")    # pre-scaled by 1/8
    WVK = nc.dram_tensor("WVK", [DM, P], BF16, kind="ExternalInput")   # [Wv | Wk]
    WO = nc.dram_tensor("WO", [DM, DM], BF16, kind="ExternalInput")
    OUT = nc.dram_tensor("OUT", [CH, DM], F32, kind="ExternalOutput")

    with tile.TileContext(nc) as tc, ExitStack() as ctx:
        perm = ctx.enter_context(tc.tile_pool(name="perm", bufs=1))

        identb = perm.tile([64, 64], F32, tag="identb")
        make_identity(nc, identb[:])
        # One-hot selector for the zinv broadcast matmul (host constant).
        # Head h<14 lives at partition h; heads 14,15 at partitions 32,33 so
        # the second (post-pair-7) reciprocal batch starts at an aligned base.
        esel = perm.tile([34, NH * 64], BF16, tag="esel")
        nc.sync.dma_start(esel[:], ESEL.ap()[:, :])

        # --- persistent SBUF tiles (all bf16)
        wvk = [perm.tile([P, P], BF16, tag=f"wvk{f}", name=f"wvk{f}") for f in range(NF)]
        wq = [perm.tile([P, DM], BF16, tag=f"wq{f}", name=f"wq{f}") for f in range(NF)]
        wo = [perm.tile([P, DM], BF16, tag=f"wo{f}", name=f"wo{f}") for f in range(NF)]
        k3T2 = perm.tile([P, YW], BF16, tag="k3T2")
        vTs = perm.tile([64, YW], F32, tag="vTs")
        v65 = [perm.tile([P, 65], BF16, tag=f"v65_{t}", name=f"v65_{t}") for t in range(NY)]
        qpT = [perm.tile([P, CH], BF16, tag=f"qpT{m}", name=f"qpT{m}") for m in range(NF)]
        ctxn = [perm.tile([P, CH], BF16, tag=f"ctxn{i}", name=f"ctxn{i}") for i in range(NPAIR)]
        cxs = [perm.tile([64, W], BF16, tag=f"cxs{h}", name=f"cxs{h}") for h in range(NH)]
        zr16 = perm.tile([34, W], F32, tag="zr16")
        zi16 = perm.tile([34, W], F32, tag="zi16")
        zi16b = perm.tile([34, W], BF16, tag="zi16b")

        for f in range(NF):
            nc.sync.dma_start(wvk[f][:], WVK.ap()[P * f:P * (f + 1), :])

        with tc.tile_pool(name="kvt", bufs=1) as kvtp, \
             tc.tile_pool(name="ph0ps", bufs=3, space="PSUM") as ph0, \
             tc.tile_pool(name="tpps", bufs=2, space="PSUM") as tpp:
            kvt = [kvtp.tile([P, YW], BF16, tag=f"kvt{f}", name=f"kvt{f}") for f in range(NF)]
            # split loads per w-chunk so the first kv-proj matmuls start early
            for n in range(3):
                for f in range(NF):
                    ns_ = slice(W * n, W * (n + 1))
                    nc.sync.dma_start(kvt[f][:, ns_], KVT.ap()[P * f:P * (f + 1), ns_])
            # kv projection: [128,512] psum per n-tile; rows 0:64=vT, 64:128=kT
            for n in range(3):
                ps = ph0.tile([P, W], F32, tag="kvp")
                for f in range(NF):
                    nc.tensor.matmul(ps[:], wvk[f][:], kvt[f][:, W * n:W * (n + 1)],
                                     start=(f == 0), stop=(f == NF - 1))
                ns = slice(W * n, W * (n + 1))
                with nc.allow_low_precision(reason="bf16 attention pipeline"):
                    nc.vector.tensor_copy(vTs[:, ns], ps[0:64, :])
                    nc.vector.tensor_copy(k3T2[64:128, ns], ps[64:128, :])
            # (vTs stays f32: the PE transpose requires out dtype == in dtype)
            # duplicate kT into the low partition half (partition remap DMA)
            nc.sync.dma_start(k3T2[0:64, :], k3T2[64:128, :])
            # v65 tiles: PE transpose of vT + ones column
            for t in range(NY):
                tp = tpp.tile([P, 64], F32, tag="tp")
                nc.tensor.transpose(tp[:], vTs[:, P * t:P * (t + 1)], identb[:])
                with nc.allow_low_precision(reason="bf16 attention pipeline"):
                    nc.vector.tensor_copy(v65[t][:, 0:64], tp[:])
                nc.vector.memset(v65[t][:, 64:65], 1.0)

        # --- q projection (1 psum bank, overlaps the attention phase) + attention
        with tc.tile_pool(name="qt", bufs=1) as qtp, \
             tc.tile_pool(name="qpps", bufs=1, space="PSUM") as qpp, \
             tc.tile_pool(name="scps", bufs=2, space="PSUM") as scp, \
             tc.tile_pool(name="cxps", bufs=3, space="PSUM") as cxp, \
             tc.tile_pool(name="pt", bufs=4) as ptp, \
             tc.tile_pool(name="zn", bufs=6) as znp:
            qt = [qtp.tile([P, CH], BF16, tag=f"qt{f}", name=f"qt{f}") for f in range(NF)]
            for f in range(NF):
                nc.sync.dma_start(qt[f][:], QT.ap()[P * f:P * (f + 1), :])
            for h2 in range(2):
                hs = slice(W * h2, W * (h2 + 1))
                for f in range(NF):
                    nc.sync.dma_start(wq[f][:, hs], WQ.ap()[P * f:P * (f + 1), hs])
            for m in range(NF):
                ps = qpp.tile([P, CH], F32, tag="qp")
                for f in range(NF):
                    nc.tensor.matmul(ps[:], wq[f][:, P * m:P * (m + 1)], qt[f][:],
                                     start=(f == 0), stop=(f == NF - 1))
                with nc.allow_low_precision(reason="bf16 attention pipeline"):
                    nc.vector.tensor_copy(qpT[m][:], ps[:])

            for f in range(NF):
                nc.sync.dma_start(wo[f][:], WO.ap()[P * f:P * (f + 1), :])

            # zr row index per head: heads 14,15 sit at aligned base 32 so the
            # second reciprocal batch has a legal start partition
            def zrow(hh):
                return hh if hh < 14 else 32 + (hh - 14)

            def z_normalize(heads):
                lo, hi = zrow(heads[0]), zrow(heads[-1]) + 1
                with nc.allow_low_precision(reason="softmax denom"):
                    nc.vector.reciprocal(zi16[lo:hi, :], zr16[lo:hi, :])
                    nc.vector.tensor_copy(zi16b[lo:hi, :], zi16[lo:hi, :])
                for hh in heads:
                    i, h = hh // 2, hh % 2
                    zl = zrow(hh)
                    zb = qpp.tile([P, W], F32, tag="qp")
                    nc.tensor.matmul(zb[0:64, :],
                                     esel[lo:hi, 64 * hh:64 * (hh + 1)],
                                     zi16b[lo:hi, :], start=True, stop=True)
                    if h == 0:
                        with nc.allow_low_precision(reason="bf16 ctx"):
                            nc.vector.tensor_mul(ctxn[i][0:64, :], cxs[hh][:],
                                                 zb[0:64, :])
                    else:
                        cbt = znp.tile([64, W], BF16, tag="cbt")
                        with nc.allow_low_precision(reason="bf16 ctx"):
                            nc.vector.tensor_mul(cbt[:], cxs[hh][:], zb[0:64, :])
                        nc.sync.dma_start(ctxn[i][64:128, :], cbt[:])

            # --- attention per head pair; scores for the two heads interleave
            # into one psum tile (A cols 0:512 rows-grp 0:64, B cols 512:1024
            # row-grp 64:128) so the row-tiled matmuls run concurrently
            for i in range(NPAIR):
                cxA = cxp.tile([P, W], F32, tag="cx")
                cxB = cxp.tile([P, W], F32, tag="cx")
                for y in range(NY):
                    ys = slice(P * y, P * (y + 1))
                    sc = scp.tile([P, 2 * W], F32, tag="sc")
                    nc.tensor.matmul(sc[:, 0:W], k3T2[0:64, ys],
                                     qpT[i][0:64, :], start=True, stop=True,
                                     tile_position=(0, 0))
                    nc.tensor.matmul(sc[:, W:2 * W], k3T2[64:128, ys],
                                     qpT[i][64:128, :], start=True, stop=True,
                                     tile_position=(64, 0))
                    pab = ptp.tile([P, 2 * W], BF16, tag="pt")
                    with nc.allow_low_precision(reason="bf16 probs"):
                        nc.scalar.activation(pab[:], sc[:], EXP)
                    st = (y == 0)
                    sp = (y == NY - 1)
                    nc.tensor.matmul(cxA[0:65, :], v65[y][:], pab[:, 0:W],
                                     start=st, stop=sp)
                    nc.tensor.matmul(cxB[0:65, :], v65[y][:], pab[:, W:2 * W],
                                     start=st, stop=sp)
                # stage Z row + unnormalized ctx out of PSUM (frees cx banks)
                for h, cx in ((0, cxA), (1, cxB)):
                    hh = 2 * i + h
                    zt = znp.tile([65, W], F32, tag="zt")
                    nc.vector.tensor_copy(zt[64:65, :], cx[64:65, :])
                    nc.sync.dma_start(zr16[zrow(hh):zrow(hh) + 1, :],
                                      zt[64:65, :])
                    with nc.allow_low_precision(reason="bf16 ctx"):
                        nc.vector.tensor_copy(cxs[hh][:], cx[0:64, :])
                if i == NPAIR - 2:
                    # normalize pairs 0-6 while pair 7's attention runs
                    z_normalize(list(range(14)))
            z_normalize([14, 15])

        # --- output projection: out[x,o] = sum_i ctxn[i][:,x].T @ wo[i][:,o]
        # i-outer so the first 7 pairs' matmuls don't wait on pair 7's ctxn
        with tc.tile_pool(name="opps", bufs=8, space="PSUM") as opp, \
             tc.tile_pool(name="osb", bufs=4) as osb:
            pso = [opp.tile([P, W], F32, tag="op", name=f"op{b}")
                   for b in range(8)]
            for i in range(NPAIR):
                for b, (x, o) in enumerate((x, o) for x in range(4) for o in range(2)):
                    xs = slice(P * x, P * (x + 1))
                    os_ = slice(W * o, W * (o + 1))
                    nc.tensor.matmul(pso[b][:], ctxn[i][:, xs], wo[i][:, os_],
                                     start=(i == 0), stop=(i == NPAIR - 1))
            for b, (x, o) in enumerate((x, o) for x in range(4) for o in range(2)):
                xs = slice(P * x, P * (x + 1))
                os_ = slice(W * o, W * (o + 1))
                ot = osb.tile([P, W], F32, tag="os")
                nc.scalar.copy(ot[:], pso[b][:])
                nc.sync.dma_start(OUT.ap()[xs, os_], ot[:])

    nc.compile()
    return nc


def _get_nc():
    if "nc" not in _CACHE:
        _CACHE["nc"] = _build()
    return _CACHE["nc"]


def _esel():
    import ml_dtypes
    e = np.zeros((34, NH * 64), ml_dtypes.bfloat16)
    for h in range(NH):
        r = h if h < 14 else 32 + (h - 14)
        e[r, 64 * h:64 * (h + 1)] = 1.0
    return e


def kernel(q, kv, Wq, Wkv, Wo, w=None, _trace=False):
    from concourse import bass_utils
    import ml_dtypes

    BF = ml_dtypes.bfloat16

    q = np.asarray(q, np.float32).reshape(L, DM)
    kv = np.asarray(kv, np.float32).reshape(L, DM)
    Wq = np.asarray(Wq, np.float32)
    Wkv = np.asarray(Wkv, np.float32)
    Wo = np.asarray(Wo, np.float32)

    qT = np.ascontiguousarray(q.T).astype(BF)           # [DM, L]
    kvT = np.ascontiguousarray(kv.T).astype(BF)         # [DM, L]
    WQs = np.ascontiguousarray(Wq / np.sqrt(DH)).astype(BF)   # fold 1/sqrt(d_head)
    WVK = np.ascontiguousarray(
        np.concatenate([Wkv[:, DH:], Wkv[:, :DH]], axis=1)).astype(BF)  # [Wv | Wk]
    WOb = np.ascontiguousarray(Wo).astype(BF)

    in_maps = []
    for c in range(NCORES):
        kvt_c = np.zeros((DM, YW), BF)
        lo = (c - 1) * CH
        hi = (c + 2) * CH
        src_lo, src_hi = max(lo, 0), min(hi, L)
        dst_lo = src_lo - lo
        kvt_c[:, dst_lo:dst_lo + (src_hi - src_lo)] = kvT[:, src_lo:src_hi]
        in_maps.append({
            "QT": np.ascontiguousarray(qT[:, c * CH:(c + 1) * CH]),
            "KVT": kvt_c,
            "WQ": WQs,
            "WVK": WVK,
            "WO": WOb,
            "ESEL": _esel(),
        })

    nc = _get_nc()
    res = bass_utils.run_bass_kernel_spmd(
        nc, in_maps, core_ids=list(range(NCORES)), trace=_trace)
    if _trace:
        _CACHE["last_result"] = res

    out = np.concatenate([r["OUT"] for r in res.results], axis=0)
    return out.reshape(B, L, DM).astype(np.float32)


# revision 29
# speedup vs baseline: 1.5228x; 1.0136x over previous
"""Local (windowed) attention with shared KV head — TRN2 Bass kernel.

Problem: b=1, L=4096, d_model=1024, n_head=16, d_head=64, w=512.
  qp = (q@Wq)/8; k,v = kv@Wkv; per 512-chunk attention over {prev,self,next}
  chunks with zero-padded edges (softmax includes exp(0)=1 terms for pads);
  out = ctx @ Wo.

Sharding: sequence-parallel over the 8 chunks, one chunk per NeuronCore.
Each core recomputes the K/V projection for its 3-chunk halo (no
collectives). Edge cores receive zero-filled halo slices, which reproduces
the reference's zero-padding exactly (scores 0 -> exp 1 in the softmax).

All matmuls in bf16 (1 cycle/row on the PE at 2.4 GHz vs ~1.5 for the
fp32 path, and bf16 activity keeps the HAM clock gate open). PSUM
accumulation stays fp32. Softmax exp runs on ScalarE reading PSUM
directly and writing bf16 probs to SBUF.

Per-core dataflow:
  kvp^T = [Wv|Wk]^T @ kv^T            (24 MMs)   -> vTs (rows 0:64), kT (64:128)
  k3T2  = kT duplicated to both partition halves (SBUF->SBUF DMA)
  v65   = PE-transpose(vTs) with a ones column appended   ([y,64+1] tiles)
  qp^T  = (Wq/8)^T @ q^T              (64 MMs)   -> 8 tiles [128,512] bf16
  scores: S^T[y,x] per head, row-packed pairs (2 heads in PE row halves)
  P^T   = exp(S^T) on ScalarE, PSUM->SBUF bf16, [128,1024] tiles
  ctx^T+Z = [v|1]^T @ P^T fused       (M=65: rows 0:64 ctx, row 64 = denom)
  norm  : zinv = recip_approx_fast(Z); broadcast via K=1 matmul; ctx * zinv
  out   = ctxn^T-tiles (lhsT) @ Wo    (64 MMs)   -> [512,1024] f32 -> DMA
"""

import numpy as np

B, L, DM, NH, DH, W = 1, 4096, 1024, 16, 64, 512
NCORES = 8
CH = L // NCORES        # 512 tokens per core
YW = 3 * W              # 1536 halo positions
P = 128
NF = DM // P            # 8 feature tiles
NY = YW // P            # 12 y tiles
NPAIR = NH // 2         # 8 head pairs
NGRP = NY // 2          # 6 score groups of 2 y-tiles

_CACHE = {}


def _build():
    import concourse.mybir as mybir
    import concourse.tile as tile
    from concourse import bacc
    from concourse.masks import make_identity
    from contextlib import ExitStack

    F32 = mybir.dt.float32
    BF16 = mybir.dt.bfloat16
    EXP = mybir.ActivationFunctionType.Exp

    nc = bacc.Bacc("TRN2", target_bir_lowering=False, debug=False)
    QT = nc.dram_tensor("QT", [DM, CH], BF16, kind="ExternalInput")
    ESEL = nc.dram_tensor("ESEL", [34, NH * 64], BF16, kind="ExternalInput")
    KVT = nc.dram_tensor("KVT", [DM, YW], BF16, kind="ExternalInput")
    WQ = nc.dram_tensor("WQ", [DM, DM], BF16, kind="ExternalInput")    # pre-scaled by 1/8
    WVK = nc.dram_tensor("WVK", [DM, P], BF16, kind="ExternalInput")   # [Wv | Wk]
    WO = nc.dram_tensor("WO", [DM, DM], BF16, kind="ExternalInput")
    OUT = nc.dram_tensor("OUT", [CH, DM], F32, kind="ExternalOutput")

    with tile.TileContext(nc) as tc, ExitStack() as ctx:
        perm = ctx.enter_context(tc.tile_pool(name="perm", bufs=1))

        identb = perm.tile([64, 64], F32, tag="identb")
        make_identity(nc, identb[:])
        # One-hot selector for the zinv broadcast matmul (host constant).
        # Head h<14 lives at partition h; heads 14,15 at partitions 32,33 so
        # the second (post-pair-7) reciprocal batch starts at an aligned base.
        esel = perm.tile([34, NH * 64], BF16, tag="esel")
        nc.sync.dma_start(esel[:], ESEL.ap()[:, :])

        # --- persistent SBUF tiles (all bf16)
        wvk = [perm.tile([P, P], BF16, tag=f"wvk{f}", name=f"wvk{f}") for f in range(NF)]
        wq = [perm.tile([P, DM], BF16, tag=f"wq{f}", name=f"wq{f}") for f in range(NF)]
        wo = [perm.tile([P, DM], BF16, tag=f"wo{f}", name=f"wo{f}") for f in range(NF)]
        k3T2 = perm.tile([P, YW], BF16, tag="k3T2")
        vTs = perm.tile([64, YW], F32, tag="vTs")
        v65 = [perm.tile([P, 65], BF16, tag=f"v65_{t}", name=f"v65_{t}") for t in range(NY)]
        qpT = [perm.tile([P, CH], BF16, tag=f"qpT{m}", name=f"qpT{m}") for m in range(NF)]
        ctxn = [perm.tile([P, CH], BF16, tag=f"ctxn{i}", name=f"ctxn{i}") for i in range(NPAIR)]
        cxs = [perm.tile([64, W], BF16, tag=f"cxs{h}", name=f"cxs{h}") for h in range(NH)]
        zr16 = perm.tile([34, W], F32, tag="zr16")
        zi16 = perm.tile([34, W], F32, tag="zi16")
        zi16b = perm.tile([34, W], BF16, tag="zi16b")

        # HAM warmup: ~4.5us of dense dummy matmuls while the input DMAs load,
        # so the PE clock gate is already open (2.4 GHz) when real work starts
        wtile = perm.tile([P, W], BF16, tag="wtile")
        nc.vector.memset(wtile[:], 1.0)
        with tc.tile_pool(name="wmps", bufs=1, space="PSUM") as wmp:
            wps = wmp.tile([P, W], F32, tag="wm")
            for _ in range(12):
                nc.tensor.matmul(wps[:], wtile[:, 0:P], wtile[:],
                                 start=True, stop=True)

        for f in range(NF):
            nc.sync.dma_start(wvk[f][:], WVK.ap()[P * f:P * (f + 1), :])

        with tc.tile_pool(name="kvt", bufs=1) as kvtp, \
             tc.tile_pool(name="ph0ps", bufs=3, space="PSUM") as ph0, \
             tc.tile_pool(name="tpps", bufs=2, space="PSUM") as tpp:
            kvt = [kvtp.tile([P, YW], BF16, tag=f"kvt{f}", name=f"kvt{f}") for f in range(NF)]
            # split loads per w-chunk so the first kv-proj matmuls start early
            for n in range(3):
                for f in range(NF):
                    ns_ = slice(W * n, W * (n + 1))
                    nc.sync.dma_start(kvt[f][:, ns_], KVT.ap()[P * f:P * (f + 1), ns_])
            # kv projection: [128,512] psum per n-tile; rows 0:64=vT, 64:128=kT
            for n in range(3):
                ps = ph0.tile([P, W], F32, tag="kvp")
                for f in range(NF):
                    nc.tensor.matmul(ps[:], wvk[f][:], kvt[f][:, W * n:W * (n + 1)],
                                     start=(f == 0), stop=(f == NF - 1))
                ns = slice(W * n, W * (n + 1))
                with nc.allow_low_precision(reason="bf16 attention pipeline"):
                    nc.vector.tensor_copy(vTs[:, ns], ps[0:64, :])
                    nc.vector.tensor_copy(k3T2[64:128, ns], ps[64:128, :])
            # (vTs stays f32: the PE transpose requires out dtype == in dtype)
            # duplicate kT into the low partition half (partition remap DMA)
            nc.sync.dma_start(k3T2[0:64, :], k3T2[64:128, :])
            # v65 tiles: PE transpose of vT + ones column
            for t in range(NY):
                tp = tpp.tile([P, 64], F32, tag="tp")
                nc.tensor.transpose(tp[:], vTs[:, P * t:P * (t + 1)], identb[:])
                with nc.allow_low_precision(reason="bf16 attention pipeline"):
                    nc.vector.tensor_copy(v65[t][:, 0:64], tp[:])
                nc.vector.memset(v65[t][:, 64:65], 1.0)

        # --- q projection (1 psum bank, overlaps the attention phase) + attention
        with tc.tile_pool(name="qt", bufs=1) as qtp, \
             tc.tile_pool(name="qpps", bufs=1, space="PSUM") as qpp, \
             tc.tile_pool(name="zn", bufs=6) as znp:
            qt = [qtp.tile([P, CH], BF16, tag=f"qt{f}", name=f"qt{f}") for f in range(NF)]
            for f in range(NF):
                nc.sync.dma_start(qt[f][:], QT.ap()[P * f:P * (f + 1), :])
            for h2 in range(2):
                hs = slice(W * h2, W * (h2 + 1))
                for f in range(NF):
                    nc.sync.dma_start(wq[f][:, hs], WQ.ap()[P * f:P * (f + 1), hs])
            for m in range(NF):
                ps = qpp.tile([P, CH], F32, tag="qp")
                for f in range(NF):
                    nc.tensor.matmul(ps[:], wq[f][:, P * m:P * (m + 1)], qt[f][:],
                                     start=(f == 0), stop=(f == NF - 1))
                with nc.allow_low_precision(reason="bf16 attention pipeline"):
                    nc.vector.tensor_copy(qpT[m][:], ps[:])

            for f in range(NF):
                nc.sync.dma_start(wo[f][:], WO.ap()[P * f:P * (f + 1), :])

            # zr row index per head: heads 14,15 sit at aligned base 32 so the
            # second reciprocal batch has a legal start partition
            def zrow(hh):
                return hh if hh < 14 else 32 + (hh - 14)

            def z_normalize(heads):
                lo, hi = zrow(heads[0]), zrow(heads[-1]) + 1
                with nc.allow_low_precision(reason="softmax denom"):
                    nc.vector.reciprocal(zi16[lo:hi, :], zr16[lo:hi, :])
                    nc.vector.tensor_copy(zi16b[lo:hi, :], zi16[lo:hi, :])
                for hh in heads:
                    i, h = hh // 2, hh % 2
                    zl = zrow(hh)
                    zb = qpp.tile([P, W], F32, tag="qp")
                    nc.tensor.matmul(zb[0:64, :],
                                     esel[lo:hi, 64 * hh:64 * (hh + 1)],
                                     zi16b[lo:hi, :], start=True, stop=True)
                    if h == 0:
                        with nc.allow_low_precision(reason="bf16 ctx"):
                            nc.vector.tensor_mul(ctxn[i][0:64, :], cxs[hh][:],
                                                 zb[0:64, :])
                    else:
                        cbt = znp.tile([64, W], BF16, tag="cbt")
                        with nc.allow_low_precision(reason="bf16 ctx"):
                            nc.vector.tensor_mul(cbt[:], cxs[hh][:], zb[0:64, :])
                        nc.sync.dma_start(ctxn[i][64:128, :], cbt[:])

            # --- attention per head pair; scores for the two heads interleave
            # into one psum tile (A cols 0:512 rows-grp 0:64, B cols 512:1024
            # row-grp 64:128) so the row-tiled matmuls run concurrently
            attn = ExitStack()
            scp = attn.enter_context(tc.tile_pool(name="scps", bufs=2, space="PSUM"))
            cxp = attn.enter_context(tc.tile_pool(name="cxps", bufs=3, space="PSUM"))
            ptp = attn.enter_context(tc.tile_pool(name="pt", bufs=4))
            for i in range(NPAIR):
                cxA = cxp.tile([P, W], F32, tag="cx")
                cxB = cxp.tile([P, W], F32, tag="cx")
                for y in range(NY):
                    ys = slice(P * y, P * (y + 1))
                    sc = scp.tile([P, 2 * W], F32, tag="sc")
                    nc.tensor.matmul(sc[:, 0:W], k3T2[0:64, ys],
                                     qpT[i][0:64, :], start=True, stop=True,
                                     tile_position=(0, 0))
                    nc.tensor.matmul(sc[:, W:2 * W], k3T2[64:128, ys],
                                     qpT[i][64:128, :], start=True, stop=True,
                                     tile_position=(64, 0))
                    pab = ptp.tile([P, 2 * W], BF16, tag="pt")
                    with nc.allow_low_precision(reason="bf16 probs"):
                        nc.scalar.activation(pab[:], sc[:], EXP)
                    st = (y == 0)
                    sp = (y == NY - 1)
                    nc.tensor.matmul(cxA[0:65, :], v65[y][:], pab[:, 0:W],
                                     start=st, stop=sp)
                    nc.tensor.matmul(cxB[0:65, :], v65[y][:], pab[:, W:2 * W],
                                     start=st, stop=sp)
                # stage Z row + unnormalized ctx out of PSUM (frees cx banks)
                for h, cx in ((0, cxA), (1, cxB)):
                    hh = 2 * i + h
                    zt = znp.tile([65, W], F32, tag="zt")
                    nc.vector.tensor_copy(zt[64:65, :], cx[64:65, :])
                    nc.sync.dma_start(zr16[zrow(hh):zrow(hh) + 1, :],
                                      zt[64:65, :])
                    with nc.allow_low_precision(reason="bf16 ctx"):
                        nc.vector.tensor_copy(cxs[hh][:], cx[0:64, :])
                if i == NPAIR - 2:
                    # normalize pairs 0-6 while pair 7's attention runs
                    z_normalize(list(range(14)))
            attn.close()   # release scores/ctx psum so outproj can start
            z_normalize([14, 15])

            # --- output projection, in two 4-bank halves; i-outer so the
            # first 7 pairs' matmuls don't wait on pair 7's ctxn
            with tc.tile_pool(name="opps", bufs=4, space="PSUM") as opp, \
                 tc.tile_pool(name="osb", bufs=4) as osb:
                allblk = [(x, o) for x in range(4) for o in range(2)]
                for half in range(2):
                    blocks = allblk[4 * half:4 * half + 4]
                    pso = [opp.tile([P, W], F32, tag="op", name=f"op{half}_{b}")
                           for b in range(4)]
                    for i in range(NPAIR):
                        for ps, (x, o) in zip(pso, blocks):
                            xs = slice(P * x, P * (x + 1))
                            os_ = slice(W * o, W * (o + 1))
                            nc.tensor.matmul(ps[:], ctxn[i][:, xs],
                                             wo[i][:, os_],
                                             start=(i == 0),
                                             stop=(i == NPAIR - 1))
                    for ps, (x, o) in zip(pso, blocks):
                        xs = slice(P * x, P * (x + 1))
                        os_ = slice(W * o, W * (o + 1))
                        ot = osb.tile([P, W], F32, tag="os")
                        nc.scalar.copy(ot[:], ps[:])
                        nc.sync.dma_start(OUT.ap()[xs, os_], ot[:])

    nc.compile()
    return nc


def _get_nc():
    if "nc" not in _CACHE:
        _CACHE["nc"] = _build()
    return _CACHE["nc"]


def _esel():
    import ml_dtypes
    e = np.zeros((34, NH * 64), ml_dtypes.bfloat16)
    for h in range(NH):
        r = h if h < 14 else 32 + (h - 14)
        e[r, 64 * h:64 * (h + 1)] = 1.0
    return e


def kernel(q, kv, Wq, Wkv, Wo, w=None, _trace=False):
    from concourse import bass_utils
    import ml_dtypes

    BF = ml_dtypes.bfloat16

    q = np.asarray(q, np.float32).reshape(L, DM)
    kv = np.asarray(kv, np.float32).reshape(L, DM)
    Wq = np.asarray(Wq, np.float32)
    Wkv = np.asarray(Wkv, np.float32)
    Wo = np.asarray(Wo, np.float32)

    qT = np.ascontiguousarray(q.T).astype(BF)           # [DM, L]
    kvT = np.ascontiguousarray(kv.T).astype(BF)         # [DM, L]
    WQs = np.ascontiguousarray(Wq / np.sqrt(DH)).astype(BF)   # fold 1/sqrt(d_head)
    WVK = np.ascontiguousarray(
        np.concatenate([Wkv[:, DH:], Wkv[:, :DH]], axis=1)).astype(BF)  # [Wv | Wk]
    WOb = np.ascontiguousarray(Wo).astype(BF)

    in_maps = []
    for c in range(NCORES):
        kvt_c = np.zeros((DM, YW), BF)
        lo = (c - 1) * CH
        hi = (c + 2) * CH
        src_lo, src_hi = max(lo, 0), min(hi, L)
        dst_lo = src_lo - lo
        kvt_c[:, dst_lo:dst_lo + (src_hi - src_lo)] = kvT[:, src_lo:src_hi]
        in_maps.append({
            "QT": np.ascontiguousarray(qT[:, c * CH:(c + 1) * CH]),
            "KVT": kvt_c,
            "WQ": WQs,
            "WVK": WVK,
            "WO": WOb,
            "ESEL": _esel(),
        })

    nc = _get_nc()
    res = bass_utils.run_bass_kernel_spmd(
        nc, in_maps, core_ids=list(range(NCORES)), trace=_trace)
    if _trace:
        _CACHE["last_result"] = res

    out = np.concatenate([r["OUT"] for r in res.results], axis=0)
    return out.reshape(B, L, DM).astype(np.float32)
